# revision 4
# baseline (speedup 1.0000x reference)
"""Trainium2 Bass kernel: per-channel 256-bin normalized histogram.

Input: full inputs [64, 512, 512, 3] float32 in [0, 1).
Output: [256, 3] float32 — per-channel histogram normalized to sum 1.

Strategy (8 NeuronCores, data-parallel over batch):
  Each core gets 8 batches = 6,291,456 elements laid out [128, 49152]
  (partition p holds 16384 consecutive pixels, channel-interleaved).

  Per core:
    1. Prep (VectorE): exact bin index idx = floor(x*256) via the
       fp32 magic-number round (y + 2^23 - 2^23) plus a compare-fix,
       written channel-separated as bf16 [128, 3, 16384] in SBUF.
    2. Count: 256 bins x 3 channels fused count passes, split across
       engines:
         - VectorE: tensor_scalar(is_equal b, reduce add) -> per-partition
           count of idx == b, one instruction per (channel, bin).
         - ScalarE: activation(Sign, bias=0.5-b, accum add) -> per-partition
           A[b] = 2*#{idx >= b} - N (a CDF); counts recovered on host via
           first differences.
    3. DMA per-partition accumulators to HBM.

  Host: sums accumulators (exact integers in fp64), all-reduces the 8
  cores' counts, and applies the per-channel fp32 normalization divide.

All counting is exact (integer counts in fp32 accumulators < 2^24), so the
result matches the reference bit-for-bit up to the final fp32 divide.
"""

import os

import numpy as np

import concourse.bacc as bacc
import concourse.bass as bass  # noqa: F401  (engine types / helpers)
import concourse.mybir as mybir
from concourse.bass_utils import run_bass_kernel_spmd
from concourse.tile import TileContext

# Problem constants (hardcoded per contract)
B, H, W, C = 64, 512, 512, 3
NBINS = 256
NCORES = 8
P = 128

BPC = B // NCORES                     # 8 batches per core
EPC = BPC * H * W * C                 # 6,291,456 elements per core
ROW = EPC // P                        # 49,152 fp32 per partition
PIXROW = ROW // C                     # 16,384 per channel per partition
CHUNK = 3072                          # fp32 per partition per prep chunk
NCHUNK = ROW // CHUNK                 # 16
CPIX = CHUNK // C                     # 1024 pixels per chunk per partition

# Bin split between engines: bins [0, D) on VectorE, [D, 256) on ScalarE
D = 154
NACT = NBINS - D

MAGIC = float(np.float32(2.0 ** 23))

_CACHE: dict = {}


def _build_module():
    nc = bacc.Bacc("TRN2", target_bir_lowering=False, debug=False,
                   num_devices=NCORES)

    x_ext = nc.declare_dram_parameter("x", [P, ROW], mybir.dt.float32,
                                      isOutput=False)
    bias_ext = nc.declare_dram_parameter("bias_tab", [P, NBINS],
                                         mybir.dt.float32, isOutput=False)
    accd_ext = nc.declare_dram_parameter("acc_dve", [P, C * D],
                                         mybir.dt.float32, isOutput=True)
    acca_ext = nc.declare_dram_parameter("acc_act", [P, C * NACT],
                                         mybir.dt.float32, isOutput=True)

    with TileContext(nc) as tc:
        with tc.tile_pool(name="persist", bufs=1) as pp:
            idx = pp.tile([P, C, PIXROW], mybir.dt.bfloat16, tag="idx")
            acc_dve = pp.tile([P, C * D], mybir.dt.float32, tag="accd")
            acc_act = pp.tile([P, C * NACT], mybir.dt.float32, tag="acca")
            bias_tab = pp.tile([P, NBINS], mybir.dt.float32, tag="bias")

            nc.sync.dma_start(out=bias_tab[:], in_=bias_ext.ap())

            # ---- Phase 1: prep (chunked) ----
            with tc.tile_pool(name="prep", bufs=2) as prep:
                for k in range(NCHUNK):
                    stage = prep.tile([P, CHUNK], mybir.dt.float32,
                                      tag="stage")
                    tsc = prep.tile([P, CHUNK], mybir.dt.float32, tag="tsc")
                    nc.sync.dma_start(
                        out=stage[:],
                        in_=x_ext.ap()[:, k * CHUNK:(k + 1) * CHUNK])
                    # y = min(x*256, 255.5)   (in place)
                    nc.vector.tensor_scalar(
                        stage[:], stage[:], 256.0, 255.5,
                        mybir.AluOpType.mult, mybir.AluOpType.min)
                    # t = (y + M) - M : round-to-nearest-even integer
                    nc.vector.tensor_scalar(
                        tsc[:], stage[:], MAGIC, -MAGIC,
                        mybir.AluOpType.add, mybir.AluOpType.add)
                    # g = t > y  (overwrites y in place)
                    nc.vector.scalar_tensor_tensor(
                        stage[:], tsc[:], 0.0, stage[:],
                        mybir.AluOpType.bypass, mybir.AluOpType.is_gt)
                    # idx_c = t - g, channel-split, bf16
                    for c in range(C):
                        nc.vector.scalar_tensor_tensor(
                            idx[:, c, k * CPIX:(k + 1) * CPIX],
                            stage[:, c::C], -1.0, tsc[:, c::C],
                            mybir.AluOpType.mult, mybir.AluOpType.add)

            # ---- Phase 2: fused count passes ----
            with tc.tile_pool(name="pass", bufs=1) as psp:
                scr_dve = psp.tile([P, PIXROW], mybir.dt.bfloat16, tag="sd")
                scr_act = psp.tile([P, PIXROW], mybir.dt.bfloat16, tag="sa")
                for c in range(C):
                    for b in range(D):
                        nc.vector.tensor_scalar(
                            scr_dve[:], idx[:, c, :], float(b), None,
                            mybir.AluOpType.is_equal, mybir.AluOpType.add,
                            accum_out=acc_dve[:, c * D + b:c * D + b + 1])
                    for b in range(D, NBINS):
                        j = c * NACT + (b - D)
                        nc.scalar.activation(
                            scr_act[:], idx[:, c, :],
                            mybir.ActivationFunctionType.Sign,
                            bias=bias_tab[:, b:b + 1], scale=1.0,
                            accum_out=acc_act[:, j:j + 1])

            # ---- Phase 3: results out ----
            nc.sync.dma_start(out=accd_ext.ap(), in_=acc_dve[:])
            nc.sync.dma_start(out=acca_ext.ap(), in_=acc_act[:])

    nc.finalize()
    return nc


def _get_module():
    if "nc" not in _CACHE:
        _CACHE["nc"] = _build_module()
    return _CACHE["nc"]


def run(x: np.ndarray, trace: bool = False):
    """x: [64, 512, 512, 3] f32. Returns ([256,3] f32, BassKernelResults)."""
    nc = _get_module()

    x = np.ascontiguousarray(x, dtype=np.float32)
    assert x.shape == (B, H, W, C)
    shards = x.reshape(NCORES, P, ROW)

    bias_tab = np.tile((0.5 - np.arange(NBINS, dtype=np.float32))[None, :],
                       (P, 1))
    in_maps = [{"x": shards[i], "bias_tab": bias_tab} for i in range(NCORES)]

    res = run_bass_kernel_spmd(nc, in_maps, list(range(NCORES)), trace=trace)

    # Host finalization: exact integer accumulation in fp64.
    counts = np.zeros((C, NBINS), dtype=np.float64)
    a_tot = np.zeros((C, NACT), dtype=np.float64)
    for r in res.results:
        ad = r["acc_dve"].astype(np.float64)          # [P, C*D]
        aa = r["acc_act"].astype(np.float64)          # [P, C*NACT]
        counts[:, :D] += ad.sum(axis=0).reshape(C, D)
        a_tot += aa.sum(axis=0).reshape(C, NACT)

    # Sign sums -> CDF: A[b] = 2*#{idx>=b} - TOT  =>  #{idx>=b} = (A+TOT)/2
    tot = float(NCORES * P * PIXROW)                  # 2^24 per channel
    s_ge = (a_tot + tot) / 2.0                        # b = D .. 255
    diff = np.empty((C, NACT), dtype=np.float64)
    diff[:, :-1] = s_ge[:, :-1] - s_ge[:, 1:]
    diff[:, -1] = s_ge[:, -1]                         # S_ge[256] == 0
    counts[:, D:] = diff

    # Normalization exactly as the reference: fp32 divide, then transpose.
    counts32 = counts.astype(np.float32)              # exact ints < 2^24
    sums = counts32.sum(axis=1, keepdims=True, dtype=np.float32)
    hist = counts32 / sums
    return np.ascontiguousarray(hist.T), res


def kernel(**inputs) -> np.ndarray:
    out, _ = run(inputs["inputs"],
                 trace=bool(os.environ.get("KERNEL_TRACE")))
    return out
